# revision 4
# baseline (speedup 1.0000x reference)
"""DIMPA 2-hop directed message passing on 8 Trainium2 NeuronCores (Bass).

Math (per direction; s uses (row=src, col=dst), t the transpose):
    deg[i] = sum_{e: row[e]=i} w[e] + FILL
    c1 = A_n x ;  c2 = A_n c1        (A_n[col,row] = w[e]/deg[row], plus
                                      self-loops (i,i) with FILL/deg[i])
    feat = w0 x + w1 c1 + w2 c2;  out = [feat_s | feat_t]

All normalization is folded into per-edge weights on the host
(wn[e] = w[e]/deg[row[e]]), so the device only runs the two sparse convs:
gather bf16 table rows by edge source (dma_gather, int16 indices over a
lo/hi-split table), build the weighted one-hot scatter matrix in bf16
(iota is_equal dl, * wn), and PSUM-accumulate 128x128 bf16 matmuls per
destination block. Host computes base = w0 x + w1 c1 between launches;
launch 2's epilogue emits w2*psum + base. Edges are grouped into chunks of
CHUNK destination blocks so each dma_gather covers ~10k rows (SWDGE
descriptor-generation overhead amortized). Two SPMD launches (hop1, hop2);
the host replicates c1 into bf16 gather tables between them.
"""

import os
import numpy as np
import ml_dtypes
from concourse import bacc, mybir
import concourse.tile as tile
from concourse.bass_utils import run_bass_kernel_spmd

FILL = 0.5
NCORES = 8
P = 128
CHUNK = 5
F32 = mybir.dt.float32
BF16 = mybir.dt.bfloat16
I16 = mybir.dt.int16
BFNP = ml_dtypes.bfloat16

LAST_EXEC_NS = []          # exec_time_ns per launch when tracing is enabled
TRACE = bool(int(os.environ.get("DIMPA_TRACE", "0")))
LAST_TRACES = []


def _execute(nc, in_maps):
    r = run_bass_kernel_spmd(nc, in_maps, list(range(NCORES)), trace=TRACE)
    if TRACE:
        LAST_EXEC_NS.append(r.exec_time_ns)
        LAST_TRACES.append(r.instructions_and_trace)
    return r.results


def _round_up(a, b):
    return (a + b - 1) // b * b


def _block_col(a):
    """[nblk*128, 128] row-major -> [128, nblk*128] block-col (node n=(b,p)
    -> [p, b*128 + f])."""
    nb = a.shape[0] // P
    return np.ascontiguousarray(
        a.reshape(nb, P, P).transpose(1, 0, 2).reshape(P, nb * P))


# ---------------------------------------------------------------- host prep

def _build_layout(row, col, wn, npad, bpc):
    """Edge layout for one direction (scatter to col blocks, gather row).

    Edges are bucketed by destination block and, within a block, by which
    half-table the source row lives in. Per-(block, half) slot counts are
    padded to the max over cores (SPMD needs identical programs) and rounded
    to 128. Packing order per core: for each chunk of CHUNK blocks, all lo
    slots (block-major), then all hi slots.

    Returns (idx_cores, dl_cores, wn_cores, caps)."""
    half = npad // 2
    nblk = npad // P

    order = np.argsort((col // P) * 2 + (row >= half), kind="stable")
    row_s = row[order]
    col_s = col[order]
    wn_s = wn[order].astype(BFNP)
    key = col_s // P * 2 + (row_s >= half)
    starts = np.searchsorted(key, np.arange(2 * nblk + 1))

    caps = []
    for jb in range(bpc):
        cl = max(starts[(c * bpc + jb) * 2 + 1] - starts[(c * bpc + jb) * 2]
                 for c in range(NCORES))
        ch = max(starts[(c * bpc + jb) * 2 + 2] - starts[(c * bpc + jb) * 2 + 1]
                 for c in range(NCORES))
        caps.append((max(_round_up(cl, P), P), max(_round_up(ch, P), P)))

    iw = sum((cl + ch) // 16 for cl, ch in caps)
    gw = sum((cl + ch) // P for cl, ch in caps)
    idx_cores, dl_cores, wn_cores = [], [], []
    for c in range(NCORES):
        idx_p = np.zeros((P, iw), dtype=np.int16)
        dl_p = np.zeros((P, gw), dtype=BFNP)
        wn_p = np.zeros((P, gw), dtype=BFNP)
        io = go = 0
        for jb0 in range(0, bpc, CHUNK):
            jb1 = min(jb0 + CHUNK, bpc)
            for ishi in (0, 1):
                for jb in range(jb0, jb1):
                    b = c * bpc + jb
                    s, e = starts[b * 2 + ishi], starts[b * 2 + ishi + 1]
                    cap = caps[jb][ishi]
                    n_e = e - s
                    r = np.zeros(cap, dtype=np.int16)
                    d = np.zeros(cap, dtype=BFNP)
                    w = np.zeros(cap, dtype=BFNP)
                    r[:n_e] = (row_s[s:e] - ishi * half).astype(np.int16)
                    d[:n_e] = (col_s[s:e] - b * P).astype(BFNP)
                    w[:n_e] = wn_s[s:e]
                    idx_p[:, io:io + cap // 16] = np.tile(
                        r.reshape(cap // 16, 16).T, (8, 1))
                    io += cap // 16
                    g = cap // P
                    dl_p[:, go:go + g] = d.reshape(g, P).T
                    wn_p[:, go:go + g] = w.reshape(g, P).T
                    go += g
        idx_cores.append(idx_p)
        dl_cores.append(dl_p)
        wn_cores.append(wn_p)
    return idx_cores, dl_cores, wn_cores, caps


# ------------------------------------------------------------- device build

def _build_launch(npad, bpc, caps_s, caps_t, mode, w2s=1.0, w2t=1.0):
    """mode 1: epilogue writes raw conv result c1 (fp32).
    mode 2: epilogue writes w2*conv + base into the [N, 2P] output."""
    half = npad // 2
    nc = bacc.Bacc(None, num_swdge_queues=4)

    iota_in = nc.declare_dram_parameter("iota", [P, P], BF16, isOutput=False)
    tabs, eg, base_in, c1_out = {}, {}, {}, {}
    for d, caps in (("s", caps_s), ("t", caps_t)):
        iw = sum((cl + ch) // 16 for cl, ch in caps)
        gw = sum((cl + ch) // P for cl, ch in caps)
        tabs[d] = (nc.declare_dram_parameter(f"tab_{d}_lo", [half, P], BF16,
                                             isOutput=False),
                   nc.declare_dram_parameter(f"tab_{d}_hi", [half, P], BF16,
                                             isOutput=False))
        eg[f"idx_{d}"] = nc.declare_dram_parameter(
            f"idx_{d}", [P, iw], I16, isOutput=False)
        eg[f"dl_{d}"] = nc.declare_dram_parameter(
            f"dl_{d}", [P, gw], BF16, isOutput=False)
        eg[f"wn_{d}"] = nc.declare_dram_parameter(
            f"wn_{d}", [P, gw], BF16, isOutput=False)
        if mode == 1:
            c1_out[d] = nc.declare_dram_parameter(
                f"c1{d}", [bpc * P, P], F32, isOutput=True)
        else:
            base_in[d] = nc.declare_dram_parameter(
                f"base_{d}", [P, bpc * P], F32, isOutput=False)
    if mode == 2:
        out = nc.declare_dram_parameter("out", [bpc * P, 2 * P], F32,
                                        isOutput=True)

    with tile.TileContext(nc) as tc:
        with (
            tc.tile_pool(name="const", bufs=1) as constp,
            tc.tile_pool(name="meta", bufs=3) as metap,
            tc.tile_pool(name="g", bufs=2) as gp,
            tc.tile_pool(name="m", bufs=2) as mp,
            tc.tile_pool(name="epi", bufs=3) as epip,
            tc.tile_pool(name="ps", bufs=4, space="PSUM") as psp,
        ):
            iota_t = constp.tile([P, 1, P], BF16)
            nc.sync.dma_start(out=iota_t[:, 0, :], in_=iota_in[:])

            for d, caps, w2, co in (("s", caps_s, w2s, 0),
                                    ("t", caps_t, w2t, P)):
                io = go = 0
                qn = 0
                for jb0 in range(0, bpc, CHUNK):
                    jb1 = min(jb0 + CHUNK, bpc)
                    nb = jb1 - jb0
                    g_lo = [caps[jb][0] // P for jb in range(jb0, jb1)]
                    g_hi = [caps[jb][1] // P for jb in range(jb0, jb1)]
                    G_lo, G_hi = sum(g_lo), sum(g_hi)
                    G = G_lo + G_hi

                    dl_t = metap.tile([P, G], BF16, tag="dl")
                    nc.sync.dma_start(out=dl_t[:], in_=eg[f"dl_{d}"][:, go:go + G])
                    wn_t = metap.tile([P, G], BF16, tag="wn")
                    nc.sync.dma_start(out=wn_t[:], in_=eg[f"wn_{d}"][:, go:go + G])
                    go += G

                    xgs = []
                    for tab, Gh in ((tabs[d][0], G_lo), (tabs[d][1], G_hi)):
                        idx_t = metap.tile([P, Gh * 8], I16, tag="idx")
                        nc.sync.dma_start(
                            out=idx_t[:], in_=eg[f"idx_{d}"][:, io:io + Gh * 8])
                        io += Gh * 8
                        xg = gp.tile([P, Gh, P], BF16, tag="xg")
                        nc.gpsimd.dma_gather(xg[:], tab[:], idx_t[:],
                                             Gh * P, Gh * P, P,
                                             single_packet=False,
                                             queue_num=qn % 4)
                        qn += 1
                        xgs.append(xg)

                    m_t = mp.tile([P, G, P], BF16, tag="m")
                    nc.vector.tensor_tensor(
                        out=m_t[:],
                        in0=iota_t[:].to_broadcast([P, G, P]),
                        in1=dl_t[:].to_broadcast([P, G, P]),
                        op=mybir.AluOpType.is_equal)
                    nc.vector.tensor_tensor(
                        out=m_t[:], in0=m_t[:],
                        in1=wn_t[:].to_broadcast([P, G, P]),
                        op=mybir.AluOpType.mult)

                    out_sb = epip.tile([P, nb, P], F32, tag="osb")
                    if mode == 2:
                        base_sb = epip.tile([P, nb, P], F32, tag="bsb")
                        nc.sync.dma_start(
                            out=base_sb[:],
                            in_=base_in[d][:, jb0 * P:jb1 * P].rearrange(
                                "p (c f) -> p c f", f=P))

                    lo_off = 0
                    hi_off = G_lo
                    for j, jb in enumerate(range(jb0, jb1)):
                        ps = psp.tile([P, P], F32, space="PSUM", tag="ps")
                        tot = g_lo[j] + g_hi[j]
                        at = 0
                        for k in range(g_lo[j]):
                            nc.tensor.matmul(
                                out=ps[:], lhsT=m_t[:, lo_off + k, :],
                                rhs=xgs[0][:, lo_off + k, :],
                                start=(at == 0), stop=(at == tot - 1))
                            at += 1
                        for k in range(g_hi[j]):
                            nc.tensor.matmul(
                                out=ps[:], lhsT=m_t[:, hi_off + k, :],
                                rhs=xgs[1][:, hi_off - G_lo + k, :],
                                start=(at == 0), stop=(at == tot - 1))
                            at += 1
                        lo_off += g_lo[j]
                        hi_off += g_hi[j]
                        if mode == 1:
                            nc.vector.tensor_scalar_add(
                                out=out_sb[:, j, :], in0=ps[:], scalar1=0.0)
                        else:
                            nc.vector.scalar_tensor_tensor(
                                out=out_sb[:, j, :], in0=ps[:],
                                scalar=float(w2), in1=base_sb[:, j, :],
                                op0=mybir.AluOpType.mult,
                                op1=mybir.AluOpType.add)

                    if mode == 1:
                        nc.sync.dma_start(
                            out=c1_out[d][jb0 * P:jb1 * P, :].rearrange(
                                "(c p) f -> p c f", p=P),
                            in_=out_sb[:])
                    else:
                        nc.sync.dma_start(
                            out=out[jb0 * P:jb1 * P, co:co + P].rearrange(
                                "(c p) f -> p c f", p=P),
                            in_=out_sb[:])

    nc.finalize()
    return nc


# ------------------------------------------------------------------ driver

def kernel(**inputs):
    x_s = np.ascontiguousarray(np.asarray(inputs["x_s"], dtype=np.float32))
    x_t = np.ascontiguousarray(np.asarray(inputs["x_t"], dtype=np.float32))
    edge_index = np.asarray(inputs["edge_index"])
    edge_weight = np.asarray(inputs["edge_weight"], dtype=np.float32)
    hop = 2
    ws = np.asarray(inputs.get("w_s", np.ones((hop + 1, 1))),
                    dtype=np.float32).ravel()
    wt = np.asarray(inputs.get("w_t", np.ones((hop + 1, 1))),
                    dtype=np.float32).ravel()

    n, dfeat = x_s.shape
    assert dfeat == P
    npad = _round_up(n, 2 * NCORES * P)
    half = npad // 2
    bpc = npad // P // NCORES
    src = edge_index[0].astype(np.int64)
    dst = edge_index[1].astype(np.int64)

    # fold row-normalization into per-edge weights; append self-loops
    loops = np.arange(n, dtype=np.int64)
    deg_s = np.bincount(src, weights=edge_weight, minlength=n) + FILL
    deg_t = np.bincount(dst, weights=edge_weight, minlength=n) + FILL
    row_a = np.concatenate([src, loops])
    col_a = np.concatenate([dst, loops])
    w_a = np.concatenate([edge_weight, np.full(n, FILL, dtype=np.float32)])
    wn_s = (w_a / deg_s[row_a]).astype(np.float32)
    wn_t = (w_a / deg_t[col_a]).astype(np.float32)

    idx_s, dl_s, wnp_s, caps_s = _build_layout(row_a, col_a, wn_s, npad, bpc)
    idx_t, dl_t, wnp_t, caps_t = _build_layout(col_a, row_a, wn_t, npad, bpc)

    iota_np = np.tile(np.arange(P, dtype=BFNP), (P, 1))

    def tab_pair(x):
        xp = np.zeros((npad, P), dtype=BFNP)
        xp[:n] = x.astype(BFNP)
        return np.ascontiguousarray(xp[:half]), np.ascontiguousarray(xp[half:])

    def edge_map(c):
        return {
            "iota": iota_np,
            "idx_s": idx_s[c], "dl_s": dl_s[c], "wn_s": wnp_s[c],
            "idx_t": idx_t[c], "dl_t": dl_t[c], "wn_t": wnp_t[c],
        }

    # ---- launch 1: c1 = A_n x
    nc1 = _build_launch(npad, bpc, caps_s, caps_t, mode=1)
    tabs1 = {"s": tab_pair(x_s), "t": tab_pair(x_t)}
    in_maps1 = []
    for c in range(NCORES):
        m = edge_map(c)
        for d in "st":
            m[f"tab_{d}_lo"], m[f"tab_{d}_hi"] = tabs1[d]
        in_maps1.append(m)
    res1 = _execute(nc1, in_maps1)

    c1 = {d: np.concatenate([res1[c][f"c1{d}"] for c in range(NCORES)], axis=0)
          for d in "st"}

    # ---- launch 2: out = w0 x + w1 c1 + w2 (A_n c1)
    base = {}
    xpad = {"s": np.zeros((npad, P), dtype=np.float32),
            "t": np.zeros((npad, P), dtype=np.float32)}
    xpad["s"][:n] = x_s
    xpad["t"][:n] = x_t
    base["s"] = ws[0] * xpad["s"] + ws[1] * c1["s"]
    base["t"] = wt[0] * xpad["t"] + wt[1] * c1["t"]

    nc2 = _build_launch(npad, bpc, caps_s, caps_t, mode=2,
                        w2s=ws[2], w2t=wt[2])
    tabs2 = {d: (np.ascontiguousarray(c1[d][:half].astype(BFNP)),
                 np.ascontiguousarray(c1[d][half:].astype(BFNP)))
             for d in "st"}
    in_maps2 = []
    for c in range(NCORES):
        m = edge_map(c)
        for d in "st":
            m[f"tab_{d}_lo"], m[f"tab_{d}_hi"] = tabs2[d]
            m[f"base_{d}"] = _block_col(
                base[d][c * bpc * P:(c + 1) * bpc * P])
        in_maps2.append(m)
    res2 = _execute(nc2, in_maps2)

    out = np.concatenate([res2[c]["out"] for c in range(NCORES)], axis=0)
    return np.ascontiguousarray(out[:n]).astype(np.float32)


# revision 7
# speedup vs baseline: 1.4797x; 1.4797x over previous
"""DIMPA 2-hop directed message passing on 8 Trainium2 NeuronCores (Bass).

Math (per direction; s uses (row=src, col=dst), t the transpose):
    deg[i] = sum_{e: row[e]=i} w[e] + FILL
    c1 = A_n x ;  c2 = A_n c1        (A_n[col,row] = w[e]/deg[row], plus
                                      self-loops (i,i) with FILL/deg[i])
    feat = w0 x + w1 c1 + w2 c2;  out = [feat_s | feat_t]

All normalization is folded into per-edge weights on the host
(wn[e] = w[e]/deg[row[e]]), so the device only runs the two sparse convs:
gather bf16 table rows by edge source (dma_gather, int16 indices over a
lo/hi-split table), build the weighted one-hot scatter matrix in bf16
(iota is_equal dl, * wn), and PSUM-accumulate 128x128 bf16 matmuls per
destination block. Host computes base = w0 x + w1 c1 between launches;
launch 2's epilogue emits w2*psum + base. Edges are grouped into chunks of
CHUNK destination blocks so each dma_gather covers ~10k rows (SWDGE
descriptor-generation overhead amortized). Two SPMD launches (hop1, hop2);
the host replicates c1 into bf16 gather tables between them.
"""

import os
import numpy as np
import ml_dtypes
from concourse import bacc, mybir
import concourse.tile as tile
from concourse.bass_utils import run_bass_kernel_spmd

FILL = 0.5
NCORES = 8
P = 128
CHUNK = 5
F32 = mybir.dt.float32
BF16 = mybir.dt.bfloat16
I16 = mybir.dt.int16
BFNP = ml_dtypes.bfloat16

LAST_EXEC_NS = []          # exec_time_ns per launch when tracing is enabled
TRACE = bool(int(os.environ.get("DIMPA_TRACE", "0")))
LAST_TRACES = []


def _execute(nc, in_maps):
    r = run_bass_kernel_spmd(nc, in_maps, list(range(NCORES)), trace=TRACE)
    if TRACE:
        LAST_EXEC_NS.append(r.exec_time_ns)
        LAST_TRACES.append(r.instructions_and_trace)
    return r.results


def _round_up(a, b):
    return (a + b - 1) // b * b


def _block_col(a):
    """[nblk*128, 128] row-major -> [128, nblk*128] block-col (node n=(b,p)
    -> [p, b*128 + f])."""
    nb = a.shape[0] // P
    return np.ascontiguousarray(
        a.reshape(nb, P, P).transpose(1, 0, 2).reshape(P, nb * P))


# ---------------------------------------------------------------- host prep

def _build_layout(row, col, wn, npad, bpc):
    """Edge layout for one direction (scatter to col blocks, gather row).

    Edges are bucketed by destination block and, within a block, by which
    half-table the source row lives in. Per-(block, half) slot counts are
    padded to the max over cores (SPMD needs identical programs) and rounded
    to 128. Packing order per core: for each chunk of CHUNK blocks, all lo
    slots (block-major), then all hi slots.

    Returns (idx_cores, dl_cores, wn_cores, caps)."""
    half = npad // 2
    nblk = npad // P

    order = np.argsort((col // P) * 2 + (row >= half), kind="stable")
    row_s = row[order]
    col_s = col[order]
    wn_s = wn[order].astype(BFNP)
    key = col_s // P * 2 + (row_s >= half)
    starts = np.searchsorted(key, np.arange(2 * nblk + 1))

    caps = []
    for jb in range(bpc):
        cl = max(starts[(c * bpc + jb) * 2 + 1] - starts[(c * bpc + jb) * 2]
                 for c in range(NCORES))
        ch = max(starts[(c * bpc + jb) * 2 + 2] - starts[(c * bpc + jb) * 2 + 1]
                 for c in range(NCORES))
        caps.append((max(_round_up(cl, P), P), max(_round_up(ch, P), P)))

    iw = sum((cl + ch) // 16 for cl, ch in caps)
    gw = sum((cl + ch) // P for cl, ch in caps)
    idx_cores, dl_cores, wn_cores = [], [], []
    for c in range(NCORES):
        idx_p = np.zeros((P, iw), dtype=np.int16)
        dl_p = np.zeros((P, gw), dtype=BFNP)
        wn_p = np.zeros((P, gw), dtype=BFNP)
        io = go = 0
        for jb0 in range(0, bpc, CHUNK):
            jb1 = min(jb0 + CHUNK, bpc)
            for ishi in (0, 1):
                for jb in range(jb0, jb1):
                    b = c * bpc + jb
                    s, e = starts[b * 2 + ishi], starts[b * 2 + ishi + 1]
                    cap = caps[jb][ishi]
                    n_e = e - s
                    r = np.zeros(cap, dtype=np.int16)
                    d = np.zeros(cap, dtype=BFNP)
                    w = np.zeros(cap, dtype=BFNP)
                    r[:n_e] = (row_s[s:e] - ishi * half).astype(np.int16)
                    d[:n_e] = (col_s[s:e] - b * P).astype(BFNP)
                    w[:n_e] = wn_s[s:e]
                    idx_p[:, io:io + cap // 16] = np.tile(
                        r.reshape(cap // 16, 16).T, (8, 1))
                    io += cap // 16
                    g = cap // P
                    dl_p[:, go:go + g] = d.reshape(g, P).T
                    wn_p[:, go:go + g] = w.reshape(g, P).T
                    go += g
        idx_cores.append(idx_p)
        dl_cores.append(dl_p)
        wn_cores.append(wn_p)
    return idx_cores, dl_cores, wn_cores, caps


# ------------------------------------------------------------- device build

def _build_launch(npad, bpc, caps_s, caps_t, mode, w2s=1.0, w2t=1.0):
    """mode 1: epilogue writes raw conv result c1 (fp32).
    mode 2: epilogue writes w2*conv + base into the [N, 2P] output."""
    half = npad // 2
    nc = bacc.Bacc(None, num_swdge_queues=4)

    iota_in = nc.declare_dram_parameter("iota", [P, P], BF16, isOutput=False)
    tabs, eg, base_in, c1_out = {}, {}, {}, {}
    for d, caps in (("s", caps_s), ("t", caps_t)):
        iw = sum((cl + ch) // 16 for cl, ch in caps)
        gw = sum((cl + ch) // P for cl, ch in caps)
        tabs[d] = (nc.declare_dram_parameter(f"tab_{d}_lo", [half, P], BF16,
                                             isOutput=False),
                   nc.declare_dram_parameter(f"tab_{d}_hi", [half, P], BF16,
                                             isOutput=False))
        eg[f"idx_{d}"] = nc.declare_dram_parameter(
            f"idx_{d}", [P, iw], I16, isOutput=False)
        eg[f"dl_{d}"] = nc.declare_dram_parameter(
            f"dl_{d}", [P, gw], BF16, isOutput=False)
        eg[f"wn_{d}"] = nc.declare_dram_parameter(
            f"wn_{d}", [P, gw], BF16, isOutput=False)
        if mode == 1:
            c1_out[d] = nc.declare_dram_parameter(
                f"c1{d}", [bpc * P, P], F32, isOutput=True)
        else:
            base_in[d] = nc.declare_dram_parameter(
                f"base_{d}", [P, bpc * P], F32, isOutput=False)
    if mode == 2:
        out = nc.declare_dram_parameter("out", [bpc * P, 2 * P], F32,
                                        isOutput=True)

    with tile.TileContext(nc) as tc:
        with (
            tc.tile_pool(name="const", bufs=1) as constp,
            tc.tile_pool(name="meta", bufs=3) as metap,
            tc.tile_pool(name="g", bufs=2) as gp,
            tc.tile_pool(name="m", bufs=2) as mp,
            tc.tile_pool(name="epi", bufs=3) as epip,
            tc.tile_pool(name="ps", bufs=4, space="PSUM") as psp,
        ):
            iota_t = constp.tile([P, 1, P], BF16)
            nc.sync.dma_start(out=iota_t[:, 0, :], in_=iota_in[:])

            dirs = (("s", caps_s, w2s, 0), ("t", caps_t, w2t, P))
            st = {d: {"io": 0, "go": 0} for d, _, _, _ in dirs}
            qn = 0
            for jb0 in range(0, bpc, CHUNK):
                for d, caps, w2, co in dirs:
                    io, go = st[d]["io"], st[d]["go"]
                    jb1 = min(jb0 + CHUNK, bpc)
                    nb = jb1 - jb0
                    g_lo = [caps[jb][0] // P for jb in range(jb0, jb1)]
                    g_hi = [caps[jb][1] // P for jb in range(jb0, jb1)]
                    G_lo, G_hi = sum(g_lo), sum(g_hi)
                    G = G_lo + G_hi

                    dl_t = metap.tile([P, G], BF16, tag="dl")
                    nc.sync.dma_start(out=dl_t[:], in_=eg[f"dl_{d}"][:, go:go + G])
                    wn_t = metap.tile([P, G], BF16, tag="wn")
                    nc.sync.dma_start(out=wn_t[:], in_=eg[f"wn_{d}"][:, go:go + G])
                    go += G

                    xgs = []
                    for tab, Gh, gl in ((tabs[d][0], G_lo, g_lo),
                                        (tabs[d][1], G_hi, g_hi)):
                        idx_t = metap.tile([P, Gh * 8], I16, tag="idx")
                        nc.sync.dma_start(
                            out=idx_t[:], in_=eg[f"idx_{d}"][:, io:io + Gh * 8])
                        io += Gh * 8
                        xg = gp.tile([P, Gh, P], BF16, tag="xg")
                        # one gather per dst block: keeps each descriptor
                        # batch inside the SWDGE ring so 4 queues overlap
                        off = 0
                        for g_b in gl:
                            nc.gpsimd.dma_gather(
                                xg[:, off:off + g_b, :], tab[:],
                                idx_t[:, off * 8:(off + g_b) * 8],
                                g_b * P, g_b * P, P,
                                single_packet=False,
                                queue_num=qn % 4)
                            qn += 1
                            off += g_b
                        xgs.append(xg)

                    m_t = mp.tile([P, G, P], BF16, tag="m")
                    nc.vector.tensor_tensor(
                        out=m_t[:],
                        in0=iota_t[:].to_broadcast([P, G, P]),
                        in1=dl_t[:].to_broadcast([P, G, P]),
                        op=mybir.AluOpType.is_equal)
                    nc.vector.tensor_tensor(
                        out=m_t[:], in0=m_t[:],
                        in1=wn_t[:].to_broadcast([P, G, P]),
                        op=mybir.AluOpType.mult)

                    out_sb = epip.tile([P, nb, P], F32, tag="osb")
                    if mode == 2:
                        base_sb = epip.tile([P, nb, P], F32, tag="bsb")
                        nc.sync.dma_start(
                            out=base_sb[:],
                            in_=base_in[d][:, jb0 * P:jb1 * P].rearrange(
                                "p (c f) -> p c f", f=P))

                    lo_off = 0
                    hi_off = G_lo
                    for j, jb in enumerate(range(jb0, jb1)):
                        ps = psp.tile([P, P], F32, space="PSUM", tag="ps")
                        tot = g_lo[j] + g_hi[j]
                        at = 0
                        for k in range(g_lo[j]):
                            nc.tensor.matmul(
                                out=ps[:], lhsT=m_t[:, lo_off + k, :],
                                rhs=xgs[0][:, lo_off + k, :],
                                start=(at == 0), stop=(at == tot - 1))
                            at += 1
                        for k in range(g_hi[j]):
                            nc.tensor.matmul(
                                out=ps[:], lhsT=m_t[:, hi_off + k, :],
                                rhs=xgs[1][:, hi_off - G_lo + k, :],
                                start=(at == 0), stop=(at == tot - 1))
                            at += 1
                        lo_off += g_lo[j]
                        hi_off += g_hi[j]
                        if mode == 1:
                            nc.vector.tensor_scalar_add(
                                out=out_sb[:, j, :], in0=ps[:], scalar1=0.0)
                        else:
                            nc.vector.scalar_tensor_tensor(
                                out=out_sb[:, j, :], in0=ps[:],
                                scalar=float(w2), in1=base_sb[:, j, :],
                                op0=mybir.AluOpType.mult,
                                op1=mybir.AluOpType.add)

                    if mode == 1:
                        nc.sync.dma_start(
                            out=c1_out[d][jb0 * P:jb1 * P, :].rearrange(
                                "(c p) f -> p c f", p=P),
                            in_=out_sb[:])
                    else:
                        nc.sync.dma_start(
                            out=out[jb0 * P:jb1 * P, co:co + P].rearrange(
                                "(c p) f -> p c f", p=P),
                            in_=out_sb[:])
                    st[d]["io"], st[d]["go"] = io, go

    nc.finalize()
    return nc


# ------------------------------------------------------------------ driver

def kernel(**inputs):
    x_s = np.ascontiguousarray(np.asarray(inputs["x_s"], dtype=np.float32))
    x_t = np.ascontiguousarray(np.asarray(inputs["x_t"], dtype=np.float32))
    edge_index = np.asarray(inputs["edge_index"])
    edge_weight = np.asarray(inputs["edge_weight"], dtype=np.float32)
    hop = 2
    ws = np.asarray(inputs.get("w_s", np.ones((hop + 1, 1))),
                    dtype=np.float32).ravel()
    wt = np.asarray(inputs.get("w_t", np.ones((hop + 1, 1))),
                    dtype=np.float32).ravel()

    n, dfeat = x_s.shape
    assert dfeat == P
    npad = _round_up(n, 2 * NCORES * P)
    half = npad // 2
    bpc = npad // P // NCORES
    src = edge_index[0].astype(np.int64)
    dst = edge_index[1].astype(np.int64)

    # fold row-normalization into per-edge weights; append self-loops
    loops = np.arange(n, dtype=np.int64)
    deg_s = np.bincount(src, weights=edge_weight, minlength=n) + FILL
    deg_t = np.bincount(dst, weights=edge_weight, minlength=n) + FILL
    row_a = np.concatenate([src, loops])
    col_a = np.concatenate([dst, loops])
    w_a = np.concatenate([edge_weight, np.full(n, FILL, dtype=np.float32)])
    wn_s = (w_a / deg_s[row_a]).astype(np.float32)
    wn_t = (w_a / deg_t[col_a]).astype(np.float32)

    idx_s, dl_s, wnp_s, caps_s = _build_layout(row_a, col_a, wn_s, npad, bpc)
    idx_t, dl_t, wnp_t, caps_t = _build_layout(col_a, row_a, wn_t, npad, bpc)

    iota_np = np.tile(np.arange(P, dtype=BFNP), (P, 1))

    def tab_pair(x):
        xp = np.zeros((npad, P), dtype=BFNP)
        xp[:n] = x.astype(BFNP)
        return np.ascontiguousarray(xp[:half]), np.ascontiguousarray(xp[half:])

    def edge_map(c):
        return {
            "iota": iota_np,
            "idx_s": idx_s[c], "dl_s": dl_s[c], "wn_s": wnp_s[c],
            "idx_t": idx_t[c], "dl_t": dl_t[c], "wn_t": wnp_t[c],
        }

    # ---- launch 1: c1 = A_n x
    nc1 = _build_launch(npad, bpc, caps_s, caps_t, mode=1)
    tabs1 = {"s": tab_pair(x_s), "t": tab_pair(x_t)}
    in_maps1 = []
    for c in range(NCORES):
        m = edge_map(c)
        for d in "st":
            m[f"tab_{d}_lo"], m[f"tab_{d}_hi"] = tabs1[d]
        in_maps1.append(m)
    res1 = _execute(nc1, in_maps1)

    c1 = {d: np.concatenate([res1[c][f"c1{d}"] for c in range(NCORES)], axis=0)
          for d in "st"}

    # ---- launch 2: out = w0 x + w1 c1 + w2 (A_n c1)
    base = {}
    xpad = {"s": np.zeros((npad, P), dtype=np.float32),
            "t": np.zeros((npad, P), dtype=np.float32)}
    xpad["s"][:n] = x_s
    xpad["t"][:n] = x_t
    base["s"] = ws[0] * xpad["s"] + ws[1] * c1["s"]
    base["t"] = wt[0] * xpad["t"] + wt[1] * c1["t"]

    nc2 = _build_launch(npad, bpc, caps_s, caps_t, mode=2,
                        w2s=ws[2], w2t=wt[2])
    tabs2 = {d: (np.ascontiguousarray(c1[d][:half].astype(BFNP)),
                 np.ascontiguousarray(c1[d][half:].astype(BFNP)))
             for d in "st"}
    in_maps2 = []
    for c in range(NCORES):
        m = edge_map(c)
        for d in "st":
            m[f"tab_{d}_lo"], m[f"tab_{d}_hi"] = tabs2[d]
            m[f"base_{d}"] = _block_col(
                base[d][c * bpc * P:(c + 1) * bpc * P])
        in_maps2.append(m)
    res2 = _execute(nc2, in_maps2)

    out = np.concatenate([res2[c]["out"] for c in range(NCORES)], axis=0)
    return np.ascontiguousarray(out[:n]).astype(np.float32)


# revision 8
# speedup vs baseline: 1.4886x; 1.0061x over previous
"""DIMPA 2-hop directed message passing on 8 Trainium2 NeuronCores (Bass).

Math (per direction; s uses (row=src, col=dst), t the transpose):
    deg[i] = sum_{e: row[e]=i} w[e] + FILL
    c1 = A_n x ;  c2 = A_n c1        (A_n[col,row] = w[e]/deg[row], plus
                                      self-loops (i,i) with FILL/deg[i])
    feat = w0 x + w1 c1 + w2 c2;  out = [feat_s | feat_t]

All normalization is folded into per-edge weights on the host
(wn[e] = w[e]/deg[row[e]]), so the device only runs the two sparse convs:
gather bf16 table rows by edge source (dma_gather, int16 indices over a
lo/hi-split table), build the weighted one-hot scatter matrix in bf16
(iota is_equal dl, * wn), and PSUM-accumulate 128x128 bf16 matmuls per
destination block. Host computes base = w0 x + w1 c1 between launches;
launch 2's epilogue emits w2*psum + base. Edges are grouped into chunks of
CHUNK destination blocks so each dma_gather covers ~10k rows (SWDGE
descriptor-generation overhead amortized). Two SPMD launches (hop1, hop2);
the host replicates c1 into bf16 gather tables between them.
"""

import os
import numpy as np
import ml_dtypes
from concourse import bacc, mybir
import concourse.tile as tile
from concourse.bass_utils import run_bass_kernel_spmd

FILL = 0.5
NCORES = 8
P = 128
CHUNK = 5
F32 = mybir.dt.float32
BF16 = mybir.dt.bfloat16
I16 = mybir.dt.int16
BFNP = ml_dtypes.bfloat16

LAST_EXEC_NS = []          # exec_time_ns per launch when tracing is enabled
TRACE = bool(int(os.environ.get("DIMPA_TRACE", "0")))
LAST_TRACES = []


def _execute(nc, in_maps):
    r = run_bass_kernel_spmd(nc, in_maps, list(range(NCORES)), trace=TRACE)
    if TRACE:
        LAST_EXEC_NS.append(r.exec_time_ns)
        LAST_TRACES.append(r.instructions_and_trace)
    return r.results


def _round_up(a, b):
    return (a + b - 1) // b * b


def _block_col(a):
    """[nblk*128, 128] row-major -> [128, nblk*128] block-col (node n=(b,p)
    -> [p, b*128 + f])."""
    nb = a.shape[0] // P
    return np.ascontiguousarray(
        a.reshape(nb, P, P).transpose(1, 0, 2).reshape(P, nb * P))


# ---------------------------------------------------------------- host prep

def _build_layout(row, col, wn, npad, bpc):
    """Edge layout for one direction (scatter to col blocks, gather row).

    Edges are bucketed by destination block and, within a block, by which
    half-table the source row lives in. Per-(block, half) slot counts are
    padded to the max over cores (SPMD needs identical programs) and rounded
    to 128. Packing order per core: for each chunk of CHUNK blocks, all lo
    slots (block-major), then all hi slots.

    Returns (idx_cores, dl_cores, wn_cores, caps)."""
    half = npad // 2
    nblk = npad // P

    order = np.argsort((col // P) * 2 + (row >= half), kind="stable")
    row_s = row[order]
    col_s = col[order]
    wn_s = wn[order].astype(BFNP)
    key = col_s // P * 2 + (row_s >= half)
    starts = np.searchsorted(key, np.arange(2 * nblk + 1))

    caps = []
    for jb in range(bpc):
        cl = max(starts[(c * bpc + jb) * 2 + 1] - starts[(c * bpc + jb) * 2]
                 for c in range(NCORES))
        ch = max(starts[(c * bpc + jb) * 2 + 2] - starts[(c * bpc + jb) * 2 + 1]
                 for c in range(NCORES))
        caps.append((max(_round_up(cl, P), P), max(_round_up(ch, P), P)))

    iw = sum((cl + ch) // 16 for cl, ch in caps)
    gw = sum((cl + ch) // P for cl, ch in caps)
    idx_cores, dl_cores, wn_cores = [], [], []
    for c in range(NCORES):
        idx_p = np.zeros((P, iw), dtype=np.int16)
        dl_p = np.zeros((P, gw), dtype=BFNP)
        wn_p = np.zeros((P, gw), dtype=BFNP)
        io = go = 0
        for jb0 in range(0, bpc, CHUNK):
            jb1 = min(jb0 + CHUNK, bpc)
            for ishi in (0, 1):
                for jb in range(jb0, jb1):
                    b = c * bpc + jb
                    s, e = starts[b * 2 + ishi], starts[b * 2 + ishi + 1]
                    cap = caps[jb][ishi]
                    n_e = e - s
                    r = np.zeros(cap, dtype=np.int16)
                    d = np.zeros(cap, dtype=BFNP)
                    w = np.zeros(cap, dtype=BFNP)
                    r[:n_e] = (row_s[s:e] - ishi * half).astype(np.int16)
                    d[:n_e] = (col_s[s:e] - b * P).astype(BFNP)
                    w[:n_e] = wn_s[s:e]
                    idx_p[:, io:io + cap // 16] = np.tile(
                        r.reshape(cap // 16, 16).T, (8, 1))
                    io += cap // 16
                    g = cap // P
                    dl_p[:, go:go + g] = d.reshape(g, P).T
                    wn_p[:, go:go + g] = w.reshape(g, P).T
                    go += g
        idx_cores.append(idx_p)
        dl_cores.append(dl_p)
        wn_cores.append(wn_p)
    return idx_cores, dl_cores, wn_cores, caps


# ------------------------------------------------------------- device build

def _build_launch(npad, bpc, caps_s, caps_t, mode, w2s=1.0, w2t=1.0):
    """mode 1: epilogue writes raw conv result c1 (fp32).
    mode 2: epilogue writes w2*conv + base into the [N, 2P] output."""
    half = npad // 2
    nc = bacc.Bacc(None, num_swdge_queues=4)

    iota_in = nc.declare_dram_parameter("iota", [P, P], BF16, isOutput=False)
    tabs, eg, base_in, c1_out = {}, {}, {}, {}
    for d, caps in (("s", caps_s), ("t", caps_t)):
        iw = sum((cl + ch) // 16 for cl, ch in caps)
        gw = sum((cl + ch) // P for cl, ch in caps)
        tabs[d] = (nc.declare_dram_parameter(f"tab_{d}_lo", [half, P], BF16,
                                             isOutput=False),
                   nc.declare_dram_parameter(f"tab_{d}_hi", [half, P], BF16,
                                             isOutput=False))
        eg[f"idx_{d}"] = nc.declare_dram_parameter(
            f"idx_{d}", [P, iw], I16, isOutput=False)
        eg[f"dl_{d}"] = nc.declare_dram_parameter(
            f"dl_{d}", [P, gw], BF16, isOutput=False)
        eg[f"wn_{d}"] = nc.declare_dram_parameter(
            f"wn_{d}", [P, gw], BF16, isOutput=False)
        if mode == 1:
            c1_out[d] = nc.declare_dram_parameter(
                f"c1{d}", [bpc * P, P], F32, isOutput=True)
        else:
            base_in[d] = nc.declare_dram_parameter(
                f"base_{d}", [P, bpc * P], F32, isOutput=False)
    if mode == 2:
        out = nc.declare_dram_parameter("out", [bpc * P, 2 * P], F32,
                                        isOutput=True)

    with tile.TileContext(nc) as tc:
        with (
            tc.tile_pool(name="const", bufs=1) as constp,
            tc.tile_pool(name="meta", bufs=3) as metap,
            tc.tile_pool(name="g", bufs=2) as gp,
            tc.tile_pool(name="m", bufs=2) as mp,
            tc.tile_pool(name="epi", bufs=3) as epip,
            tc.tile_pool(name="ps", bufs=4, space="PSUM") as psp,
        ):
            iota_t = constp.tile([P, 1, P], BF16)
            nc.sync.dma_start(out=iota_t[:, 0, :], in_=iota_in[:])

            dirs = (("s", caps_s, w2s, 0), ("t", caps_t, w2t, P))
            st = {d: {"io": 0, "go": 0} for d, _, _, _ in dirs}
            qn = 0
            for jb0 in range(0, bpc, CHUNK):
                for d, caps, w2, co in dirs:
                    io, go = st[d]["io"], st[d]["go"]
                    jb1 = min(jb0 + CHUNK, bpc)
                    nb = jb1 - jb0
                    g_lo = [caps[jb][0] // P for jb in range(jb0, jb1)]
                    g_hi = [caps[jb][1] // P for jb in range(jb0, jb1)]
                    G_lo, G_hi = sum(g_lo), sum(g_hi)
                    G = G_lo + G_hi

                    dl_t = metap.tile([P, G], BF16, tag="dl")
                    nc.sync.dma_start(out=dl_t[:], in_=eg[f"dl_{d}"][:, go:go + G])
                    wn_t = metap.tile([P, G], BF16, tag="wn")
                    nc.sync.dma_start(out=wn_t[:], in_=eg[f"wn_{d}"][:, go:go + G])
                    go += G

                    xgs = []
                    for tab, Gh, gl in ((tabs[d][0], G_lo, g_lo),
                                        (tabs[d][1], G_hi, g_hi)):
                        idx_t = metap.tile([P, Gh * 8], I16, tag="idx")
                        nc.sync.dma_start(
                            out=idx_t[:], in_=eg[f"idx_{d}"][:, io:io + Gh * 8])
                        io += Gh * 8
                        xg = gp.tile([P, Gh, P], BF16, tag="xg")
                        # one gather per pair of dst blocks: big enough to
                        # amortize the ~1us SWDGE fixed cost, small enough
                        # to stay inside the ring so 4 queues overlap
                        grp = [sum(gl[i:i + 2]) for i in range(0, len(gl), 2)]
                        off = 0
                        for g_b in grp:
                            nc.gpsimd.dma_gather(
                                xg[:, off:off + g_b, :], tab[:],
                                idx_t[:, off * 8:(off + g_b) * 8],
                                g_b * P, g_b * P, P,
                                single_packet=False,
                                queue_num=qn % 4)
                            qn += 1
                            off += g_b
                        xgs.append(xg)

                    m_t = mp.tile([P, G, P], BF16, tag="m")
                    nc.vector.tensor_tensor(
                        out=m_t[:],
                        in0=iota_t[:].to_broadcast([P, G, P]),
                        in1=dl_t[:].to_broadcast([P, G, P]),
                        op=mybir.AluOpType.is_equal)
                    nc.vector.tensor_tensor(
                        out=m_t[:], in0=m_t[:],
                        in1=wn_t[:].to_broadcast([P, G, P]),
                        op=mybir.AluOpType.mult)

                    out_sb = epip.tile([P, nb, P], F32, tag="osb")
                    if mode == 2:
                        base_sb = epip.tile([P, nb, P], F32, tag="bsb")
                        nc.sync.dma_start(
                            out=base_sb[:],
                            in_=base_in[d][:, jb0 * P:jb1 * P].rearrange(
                                "p (c f) -> p c f", f=P))

                    lo_off = 0
                    hi_off = G_lo
                    for j, jb in enumerate(range(jb0, jb1)):
                        ps = psp.tile([P, P], F32, space="PSUM", tag="ps")
                        tot = g_lo[j] + g_hi[j]
                        at = 0
                        for k in range(g_lo[j]):
                            nc.tensor.matmul(
                                out=ps[:], lhsT=m_t[:, lo_off + k, :],
                                rhs=xgs[0][:, lo_off + k, :],
                                start=(at == 0), stop=(at == tot - 1))
                            at += 1
                        for k in range(g_hi[j]):
                            nc.tensor.matmul(
                                out=ps[:], lhsT=m_t[:, hi_off + k, :],
                                rhs=xgs[1][:, hi_off - G_lo + k, :],
                                start=(at == 0), stop=(at == tot - 1))
                            at += 1
                        lo_off += g_lo[j]
                        hi_off += g_hi[j]
                        if mode == 1:
                            nc.vector.tensor_scalar_add(
                                out=out_sb[:, j, :], in0=ps[:], scalar1=0.0)
                        else:
                            nc.vector.scalar_tensor_tensor(
                                out=out_sb[:, j, :], in0=ps[:],
                                scalar=float(w2), in1=base_sb[:, j, :],
                                op0=mybir.AluOpType.mult,
                                op1=mybir.AluOpType.add)

                    if mode == 1:
                        nc.sync.dma_start(
                            out=c1_out[d][jb0 * P:jb1 * P, :].rearrange(
                                "(c p) f -> p c f", p=P),
                            in_=out_sb[:])
                    else:
                        nc.sync.dma_start(
                            out=out[jb0 * P:jb1 * P, co:co + P].rearrange(
                                "(c p) f -> p c f", p=P),
                            in_=out_sb[:])
                    st[d]["io"], st[d]["go"] = io, go

    nc.finalize()
    return nc


# ------------------------------------------------------------------ driver

def kernel(**inputs):
    x_s = np.ascontiguousarray(np.asarray(inputs["x_s"], dtype=np.float32))
    x_t = np.ascontiguousarray(np.asarray(inputs["x_t"], dtype=np.float32))
    edge_index = np.asarray(inputs["edge_index"])
    edge_weight = np.asarray(inputs["edge_weight"], dtype=np.float32)
    hop = 2
    ws = np.asarray(inputs.get("w_s", np.ones((hop + 1, 1))),
                    dtype=np.float32).ravel()
    wt = np.asarray(inputs.get("w_t", np.ones((hop + 1, 1))),
                    dtype=np.float32).ravel()

    n, dfeat = x_s.shape
    assert dfeat == P
    npad = _round_up(n, 2 * NCORES * P)
    half = npad // 2
    bpc = npad // P // NCORES
    src = edge_index[0].astype(np.int64)
    dst = edge_index[1].astype(np.int64)

    # fold row-normalization into per-edge weights; append self-loops
    loops = np.arange(n, dtype=np.int64)
    deg_s = np.bincount(src, weights=edge_weight, minlength=n) + FILL
    deg_t = np.bincount(dst, weights=edge_weight, minlength=n) + FILL
    row_a = np.concatenate([src, loops])
    col_a = np.concatenate([dst, loops])
    w_a = np.concatenate([edge_weight, np.full(n, FILL, dtype=np.float32)])
    wn_s = (w_a / deg_s[row_a]).astype(np.float32)
    wn_t = (w_a / deg_t[col_a]).astype(np.float32)

    idx_s, dl_s, wnp_s, caps_s = _build_layout(row_a, col_a, wn_s, npad, bpc)
    idx_t, dl_t, wnp_t, caps_t = _build_layout(col_a, row_a, wn_t, npad, bpc)

    iota_np = np.tile(np.arange(P, dtype=BFNP), (P, 1))

    def tab_pair(x):
        xp = np.zeros((npad, P), dtype=BFNP)
        xp[:n] = x.astype(BFNP)
        return np.ascontiguousarray(xp[:half]), np.ascontiguousarray(xp[half:])

    def edge_map(c):
        return {
            "iota": iota_np,
            "idx_s": idx_s[c], "dl_s": dl_s[c], "wn_s": wnp_s[c],
            "idx_t": idx_t[c], "dl_t": dl_t[c], "wn_t": wnp_t[c],
        }

    # ---- launch 1: c1 = A_n x
    nc1 = _build_launch(npad, bpc, caps_s, caps_t, mode=1)
    tabs1 = {"s": tab_pair(x_s), "t": tab_pair(x_t)}
    in_maps1 = []
    for c in range(NCORES):
        m = edge_map(c)
        for d in "st":
            m[f"tab_{d}_lo"], m[f"tab_{d}_hi"] = tabs1[d]
        in_maps1.append(m)
    res1 = _execute(nc1, in_maps1)

    c1 = {d: np.concatenate([res1[c][f"c1{d}"] for c in range(NCORES)], axis=0)
          for d in "st"}

    # ---- launch 2: out = w0 x + w1 c1 + w2 (A_n c1)
    base = {}
    xpad = {"s": np.zeros((npad, P), dtype=np.float32),
            "t": np.zeros((npad, P), dtype=np.float32)}
    xpad["s"][:n] = x_s
    xpad["t"][:n] = x_t
    base["s"] = ws[0] * xpad["s"] + ws[1] * c1["s"]
    base["t"] = wt[0] * xpad["t"] + wt[1] * c1["t"]

    nc2 = _build_launch(npad, bpc, caps_s, caps_t, mode=2,
                        w2s=ws[2], w2t=wt[2])
    tabs2 = {d: (np.ascontiguousarray(c1[d][:half].astype(BFNP)),
                 np.ascontiguousarray(c1[d][half:].astype(BFNP)))
             for d in "st"}
    in_maps2 = []
    for c in range(NCORES):
        m = edge_map(c)
        for d in "st":
            m[f"tab_{d}_lo"], m[f"tab_{d}_hi"] = tabs2[d]
            m[f"base_{d}"] = _block_col(
                base[d][c * bpc * P:(c + 1) * bpc * P])
        in_maps2.append(m)
    res2 = _execute(nc2, in_maps2)

    out = np.concatenate([res2[c]["out"] for c in range(NCORES)], axis=0)
    return np.ascontiguousarray(out[:n]).astype(np.float32)
